# revision 17
# baseline (speedup 1.0000x reference)
"""Trainium2 Bass kernel for nn_Attention_86423331930617.

Reference math (per batch b of 16):
  frate = [framerate[b], resolution[b]]                       # [2]
  h  = ((frate@W1+b1)@W2+b2)@W3+b3                            # [98304]
  qkvw = softmax(h.reshape(128, 768), axis=0)                 # over dim d
  wq, wk, wv = split(qkvw, 3, -1)                             # [128, 256] each
  q/k/v = x[b] @ w*      -> heads [8, 600, 32]
  dots = q@k.T / sqrt(32); attn = softmax(dots, -1) * mask[b]
  out  = attn @ v -> [600, 256]
  ow   = softmax(((frate@V1+c1)@V2+c2)@V3+c3 .reshape(256,128), axis=0)
  y    = out @ ow                                             # [600, 128]

Distribution over 8 cores (single NEFF, one AllToAll):
  - Hypernet big matmuls are column-sharded: core c computes
    h[:, d in 16c..16c+16) for ALL 16 batches (reads only its 1/8 slice
    of W3/V3), then one AllToAll redistributes so core c holds the full
    h for its two batches (2c, 2c+1).
  - Attention is batch-sharded: core c does batches 2c, 2c+1.
  Softmax normalizers are folded into per-partition ACT scales.
  exp() needs no max-subtraction: hypernet outputs and dots are O(0.1)
  by construction (glorot-scaled linear chains on [0,1) inputs).
"""
import sys

sys.path.insert(0, "/opt/trn_rl_repo")
if "/root/.axon_site" not in sys.path:
    sys.path.insert(0, "/root/.axon_site")

import numpy as np
import ml_dtypes

import concourse.bass as bass
import concourse.mybir as mybir
import concourse.tile as tile
from concourse.vector_clock import ScopedClock
from concourse.bass_utils import run_bass_kernel_spmd

F32 = mybir.dt.float32
F32R = mybir.dt.float32r
BF16 = mybir.dt.bfloat16
BF16_NP = ml_dtypes.bfloat16
FP8 = mybir.dt.float8e4
FP8_NP = mybir.dt.np(mybir.dt.float8e4)
W3_SCALE = 64.0
A2_SCALE = 4.0
UNSCALE = 1.0 / (W3_SCALE * A2_SCALE)
EXP = mybir.ActivationFunctionType.Exp
IDENT = mybir.ActivationFunctionType.Identity

NCORES = 8
B, N, DIM, HEADS, DH = 16, 600, 128, 8, 32
INNER = HEADS * DH          # 256
D3 = 3 * DIM                # 384
E3 = 3 * INNER              # 768
BPC = B // NCORES           # batches per core = 2
W3_SL = 16 * E3             # 12288 w3 cols per core (16 d-rows)
V3_SL = 32 * DIM            # 4096 v3 cols per core (32 i-rows)
A2A_W = W3_SL + V3_SL       # 16384
SCALE = DH ** -0.5
NH = [(0, 300), (300, 300)]                       # n halves
MT = [(0, 128), (128, 128), (256, 128), (384, 128), (512, 88)]  # m tiles


# ---------------------------------------------------------------------------
# This walrus build accepts at most ONE sync wait / update per instruction;
# Tile can emit more. Split extras onto adjacent same-engine NoOps.
class _SplitWaitTileContext(tile.TileContext):
    def _split_sync(self, insts):
        out = []
        for inst in insts:
            si = inst.sync_info
            if si is None:
                out.append(inst)
                continue
            waits = list(si.on_wait) if si.on_wait else []
            updates = list(si.on_update) if si.on_update else []
            if len(waits) <= 1 and len(updates) <= 1:
                out.append(inst)
                continue
            for w in waits[1:]:
                nop = mybir.InstNoOp(name=f"I-{self.nc.next_id()}", ins=[], outs=[])
                nop.engine = inst.engine
                nop.sync_info = mybir.SyncInfo(on_wait=[w], on_update=[])
                out.append(nop)
            inst.sync_info = mybir.SyncInfo(on_wait=waits[:1], on_update=updates[:1])
            out.append(inst)
            for u in updates[1:]:
                nop = mybir.InstNoOp(name=f"I-{self.nc.next_id()}", ins=[], outs=[])
                nop.engine = inst.engine
                nop.sync_info = mybir.SyncInfo(on_wait=[], on_update=[u])
                out.append(nop)
        return out

    def _lower_ordered_insts(self, ordered):
        for bb_name in list(ordered.keys()):
            ordered[bb_name] = self._split_sync(ordered[bb_name])
        return super()._lower_ordered_insts(ordered)

    def _drain_and_barrier(self, tick_clock, wait_clock):
        nc = self.nc
        probe = nc.sync.nop()
        wait_clock.add_sem_waits(probe.ins, ScopedClock({None: tick_clock.global_clock}))
        si = probe.ins.sync_info
        waits = list(si.on_wait) if si is not None and si.on_wait else []
        if len(waits) > 1:
            probe.ins.sync_info = mybir.SyncInfo(on_wait=waits[:1], on_update=[])
            for w in waits[1:]:
                extra = nc.sync.nop()
                extra.ins.sync_info = mybir.SyncInfo(on_wait=[w], on_update=[])
        nc.sync.drain()
        nc.all_engine_barrier()
        assert self.sems is not None
        popped = nc._tile_sem_poison_stack.pop()
        assert popped is self._sem_poison
        nc.clear_and_free_semaphores(list(self.sems.allocated().values()))
        nc.all_engine_barrier()


# ---------------------------------------------------------------------------
def _build_program(with_bias):
    """Emit the per-core SPMD program. with_bias: (b12, c12, b3, c3) flags."""
    b12, c12, b3f, c3f = with_bias
    nc = bass.Bass("TRN2", target_bir_lowering=False, debug=False,
                   num_devices=NCORES)

    xT = nc.dram_tensor("xT", [BPC, DIM, N], BF16, kind="ExternalInput")
    maskT = nc.dram_tensor("maskT", [BPC, N, N], BF16, kind="ExternalInput")
    frateT = nc.dram_tensor("frateT", [2, B], F32, kind="ExternalInput")
    w1 = nc.dram_tensor("w1", [2, D3], F32, kind="ExternalInput")
    w2 = nc.dram_tensor("w2", [D3, D3], F32, kind="ExternalInput")
    w3c = nc.dram_tensor("w3c", [D3, W3_SL], FP8, kind="ExternalInput")
    v1 = nc.dram_tensor("v1", [2, INNER], F32, kind="ExternalInput")
    v2 = nc.dram_tensor("v2", [INNER, INNER], F32, kind="ExternalInput")
    v3c = nc.dram_tensor("v3c", [INNER, V3_SL], FP8, kind="ExternalInput")
    if b12:
        b1t = nc.dram_tensor("b1t", [D3, 1], F32, kind="ExternalInput")
        b2t = nc.dram_tensor("b2t", [D3, 1], F32, kind="ExternalInput")
    if c12:
        c1t = nc.dram_tensor("c1t", [INNER, 1], F32, kind="ExternalInput")
        c2t = nc.dram_tensor("c2t", [INNER, 1], F32, kind="ExternalInput")
    if b3f:
        b3c = nc.dram_tensor("b3c", [1, W3_SL], BF16, kind="ExternalInput")
    if c3f:
        c3c = nc.dram_tensor("c3c", [1, V3_SL], BF16, kind="ExternalInput")
    yT = nc.dram_tensor("yT", [BPC, DIM, N], F32, kind="ExternalOutput")

    with _SplitWaitTileContext(nc) as tc:
        with (
            tc.tile_pool(name="const", bufs=1) as cpool,
            tc.tile_pool(name="wts", bufs=1) as wpool,
            tc.tile_pool(name="achain", bufs=1) as apool,
            tc.tile_pool(name="hcopy", bufs=4) as hpool,
            tc.tile_pool(name="batch", bufs=1) as bpool,
            tc.tile_pool(name="attn", bufs=6) as epool,
            tc.tile_pool(name="rrsp", bufs=36) as rrs_pool,
            # PSUM budget (8 banks of 2KB/partition):
            #   pd x3 (1 bank each), po x2, prs x1, pb x1, pcs x1  = 8
            tc.tile_pool(name="psD", bufs=4, space="PSUM") as psD,
            tc.tile_pool(name="psO", bufs=2, space="PSUM") as psO,
            tc.tile_pool(name="psS", bufs=1, space="PSUM") as psS,
            tc.tile_pool(name="dram", bufs=1, space="DRAM") as dpool,
        ):
            # ---- constants
            ones_col = cpool.tile([DIM, 1], BF16, name="ones_col")
            nc.vector.memset(ones_col[:], 1.0)
            ones_row32 = cpool.tile([1, 32], BF16, name="ones_row32")
            nc.vector.memset(ones_row32[:], 1.0)
            if b3f or c3f:
                ones_row16 = cpool.tile([1, B], BF16, name="ones_row16")
                nc.vector.memset(ones_row16[:], 1.0)

            # ---- small weights in
            fr_sb = wpool.tile([2, B], F32, name="fr_sb")
            nc.sync.dma_start(out=fr_sb[:], in_=frateT[:])
            w1_sb = wpool.tile([2, D3], F32, name="w1_sb")
            nc.sync.dma_start(out=w1_sb[:], in_=w1[:])
            w2_sb = [wpool.tile([DIM, D3], F32, name=f"w2_sb{k}") for k in range(3)]
            for k in range(3):
                nc.sync.dma_start(out=w2_sb[k][:], in_=w2[128 * k:128 * (k + 1), :])
            v1_sb = wpool.tile([2, INNER], F32, name="v1_sb")
            nc.sync.dma_start(out=v1_sb[:], in_=v1[:])
            v2_sb = [wpool.tile([DIM, INNER], F32, name=f"v2_sb{k}") for k in range(2)]
            for k in range(2):
                nc.sync.dma_start(out=v2_sb[k][:], in_=v2[128 * k:128 * (k + 1), :])
            bias_sb = {}
            if b12:
                bias_sb["b1"] = wpool.tile([D3, 1], F32, name="b1_sb")
                nc.sync.dma_start(out=bias_sb["b1"][:], in_=b1t[:])
                bias_sb["b2"] = wpool.tile([D3, 1], F32, name="b2_sb")
                nc.sync.dma_start(out=bias_sb["b2"][:], in_=b2t[:])
            if c12:
                bias_sb["c1"] = wpool.tile([INNER, 1], F32, name="c1_sb")
                nc.sync.dma_start(out=bias_sb["c1"][:], in_=c1t[:])
                bias_sb["c2"] = wpool.tile([INNER, 1], F32, name="c2_sb")
                nc.sync.dma_start(out=bias_sb["c2"][:], in_=c2t[:])
            if b3f:
                b3_sb = wpool.tile([1, W3_SL], BF16, name="b3_sb")
                nc.sync.dma_start(out=b3_sb[:], in_=b3c[:])
            if c3f:
                c3_sb = wpool.tile([1, V3_SL], BF16, name="c3_sb")
                nc.sync.dma_start(out=c3_sb[:], in_=c3c[:])

            # ---- big hypernet weight slices (freed before phase B needs
            # the masked-exp tile pool)
            w3ctx = tc.tile_pool(name="w3", bufs=1)
            w3pool = w3ctx.__enter__()
            w3dr = w3pool.tile([DIM, 3, W3_SL], FP8, name="w3dr")
            w3view = w3c[:].rearrange("(ks p) n -> p ks n", p=DIM)
            for q4 in range(4):
                q0 = q4 * (W3_SL // 4)
                nc.sync.dma_start(out=w3dr[:, :, q0:q0 + W3_SL // 4],
                                  in_=w3view[:, :, q0:q0 + W3_SL // 4])
            v3dr = w3pool.tile([DIM, 2, V3_SL], FP8, name="v3dr")
            nc.sync.dma_start(
                out=v3dr[:], in_=v3c[:].rearrange("(ks p) n -> p ks n", p=DIM))

            # ---- x / mask inputs (phase B data; queued behind weights)
            xT_sb = [bpool.tile([DIM, N], BF16, name=f"xT_sb{i}")
                     for i in range(BPC)]
            for i in range(BPC):
                nc.sync.dma_start(out=xT_sb[i][:], in_=xT[i])
            maskT_sb = [[bpool.tile([128, N], BF16, name=f"mask_sb{i}_{mt}")
                         for mt in range(5)] for i in range(BPC)]

            # ---- a-chain: a1T = (frate@W1+b1).T as 3x[128,16]
            a1T = []
            for t in range(3):
                p = psD.tile([DIM, 512], F32, name="pa", tag="pd")
                nc.tensor.matmul(p[:, :B], w1_sb[:, 128 * t:128 * (t + 1)],
                                 fr_sb[:], start=True, stop=True)
                s = apool.tile([DIM, B], F32, name=f"a1T{t}")
                if b12:
                    nc.scalar.activation(s[:], p[:, :B], IDENT,
                                         bias=bias_sb["b1"][128 * t:128 * (t + 1), :])
                else:
                    nc.scalar.copy(s[:], p[:, :B])
                a1T.append(s)
            a2f8 = apool.tile([DIM, 3, B], FP8, name="a2f8")
            for t in range(3):
                p = psD.tile([DIM, 512], F32, name="pa2", tag="pd")
                for k in range(3):
                    nc.tensor.matmul(p[:, :B], w2_sb[k][:, 128 * t:128 * (t + 1)],
                                     a1T[k][:], start=(k == 0), stop=(k == 2))
                if b12:
                    tmp = apool.tile([DIM, B], F32, name=f"a2tmp{t}")
                    nc.scalar.activation(tmp[:], p[:, :B], IDENT,
                                         bias=bias_sb["b2"][128 * t:128 * (t + 1), :])
                    with nc.allow_low_precision("fp8 hypernet activations"):
                        nc.vector.tensor_scalar_mul(a2f8[:, t, :], tmp[:],
                                                    A2_SCALE)
                else:
                    nc.scalar.mul(a2f8[:, t, :], p[:, :B], A2_SCALE)
            av1T = []
            for t in range(2):
                p = psD.tile([DIM, 512], F32, name="pav", tag="pd")
                nc.tensor.matmul(p[:, :B], v1_sb[:, 128 * t:128 * (t + 1)],
                                 fr_sb[:], start=True, stop=True)
                s = apool.tile([DIM, B], F32, name=f"av1T{t}")
                if c12:
                    nc.scalar.activation(s[:], p[:, :B], IDENT,
                                         bias=bias_sb["c1"][128 * t:128 * (t + 1), :])
                else:
                    nc.scalar.copy(s[:], p[:, :B])
                av1T.append(s)
            avf8 = apool.tile([DIM, 2, B], FP8, name="avf8")
            for t in range(2):
                p = psD.tile([DIM, 512], F32, name="pav2", tag="pd")
                for k in range(2):
                    nc.tensor.matmul(p[:, :B], v2_sb[k][:, 128 * t:128 * (t + 1)],
                                     av1T[k][:], start=(k == 0), stop=(k == 1))
                if c12:
                    tmp = apool.tile([DIM, B], F32, name=f"avtmp{t}")
                    nc.scalar.activation(tmp[:], p[:, :B], IDENT,
                                         bias=bias_sb["c2"][128 * t:128 * (t + 1), :])
                    with nc.allow_low_precision("fp8 hypernet activations"):
                        nc.vector.tensor_scalar_mul(avf8[:, t, :], tmp[:],
                                                    A2_SCALE)
                else:
                    nc.scalar.mul(avf8[:, t, :], p[:, :B], A2_SCALE)

            # ---- big hypernet matmuls -> two a2a inputs
            # host reorders w3c cols: first 16x512 (d-major, e<512 = q,k),
            # then 16x256 (e>=512 = v). a2a #1 ships q/k; a2a #2 ships
            # v + ow and overlaps with the dots/softmax work.
            QK_W = 16 * 512
            VB_W = 16 * 256
            a2a_in1 = dpool.tile([B, QK_W], BF16, name="a2a_in1")
            a2a_in2 = dpool.tile([B, VB_W + V3_SL], BF16, name="a2a_in2")
            CH = 512
            def w3_chunk(j):
                p = psD.tile([B, CH], F32, name="ph", tag="pd")
                nc.tensor.matmul(p[:], a2f8[:, 0:2, :],
                                 w3dr[:, 0:2, CH * j:CH * (j + 1)],
                                 start=True, stop=False,
                                 perf_mode=mybir.MatmulPerfMode.DoubleRow)
                nc.tensor.matmul(p[:], a2f8[:, 2, :],
                                 w3dr[:, 2, CH * j:CH * (j + 1)],
                                 start=False, stop=not b3f)
                if b3f:
                    nc.tensor.matmul(p[:], ones_row16[:],
                                     b3_sb[:, CH * j:CH * (j + 1)],
                                     start=False, stop=True)
                s = hpool.tile([B, CH], BF16, name="hs", tag="hs")
                if j % 2 == 0:
                    nc.scalar.mul(s[:], p[:], UNSCALE)
                else:
                    with nc.allow_low_precision("bf16 h exchange"):
                        nc.vector.tensor_scalar_mul(s[:], p[:], UNSCALE)
                if j < 16:
                    nc.sync.dma_start(out=a2a_in1[:, CH * j:CH * (j + 1)],
                                      in_=s[:])
                else:
                    jj = j - 16
                    nc.sync.dma_start(out=a2a_in2[:, CH * jj:CH * (jj + 1)],
                                      in_=s[:])

            for j in range(16):
                w3_chunk(j)
            a2a_out1 = dpool.tile([B, QK_W], BF16, name="a2a_out1")
            nc.gpsimd.collective_compute(
                "AllToAll", mybir.AluOpType.bypass,
                replica_groups=[list(range(NCORES))],
                ins=[a2a_in1[:]], outs=[a2a_out1[:]],
            )
            # mask loads ride the a2a window instead of competing with the
            # W3 stream at startup
            for i in range(BPC):
                for mt, (m0, msz) in enumerate(MT):
                    nc.sync.dma_start(out=maskT_sb[i][mt][:msz, :],
                                      in_=maskT[i, m0:m0 + msz, :])
            for j in range(16, W3_SL // CH):
                w3_chunk(j)
            for j in range(V3_SL // CH):
                p = psD.tile([B, CH], F32, name="phv", tag="pd")
                nc.tensor.matmul(p[:], avf8[:, 0:2, :],
                                 v3dr[:, 0:2, CH * j:CH * (j + 1)],
                                 start=True, stop=not c3f,
                                 perf_mode=mybir.MatmulPerfMode.DoubleRow)
                if c3f:
                    nc.tensor.matmul(p[:], ones_row16[:],
                                     c3_sb[:, CH * j:CH * (j + 1)],
                                     start=False, stop=True)
                s = hpool.tile([B, CH], BF16, name="hvs", tag="hs")
                nc.scalar.mul(s[:], p[:], UNSCALE)
                nc.sync.dma_start(
                    out=a2a_in2[:, VB_W + CH * j:VB_W + CH * (j + 1)], in_=s[:])
            a2a_out2 = dpool.tile([B, VB_W + V3_SL], BF16, name="a2a_out2")
            nc.gpsimd.collective_compute(
                "AllToAll", mybir.AluOpType.bypass,
                replica_groups=[list(range(NCORES))],
                ins=[a2a_in2[:]], outs=[a2a_out2[:]],
            )
            w3ctx.__exit__(None, None, None)
            emctx = tc.tile_pool(name="emk", bufs=162)
            em_pool = emctx.__enter__()
            # row (2s+i) holds my batch i's hypernet outputs from source s
            h1view = a2a_out1[:].rearrange(
                "(s two) (d e) -> two s d e", two=BPC, d=16)
            h2view = a2a_out2[:, :VB_W].rearrange(
                "(s two) (d e) -> two s d e", two=BPC, d=16)
            hvview = a2a_out2[:, VB_W:].rearrange(
                "(s two) (iv dd) -> two s iv dd", two=BPC, iv=32)

            # ================= attention =================
            # part 1 for BOTH batches first (needs only a2a #1), then both
            # part 2s: the PE has ~2 batches of dots/rowsum work queued
            # before anything waits on a2a #2.
            p1_state = {}
            for i in range(BPC):
                # ---------- part 1: needs only a2a #1 (q/k) ----------
                qrawA = bpool.tile([DIM, 512], BF16, name="qrawA", tag="qrawA")
                for s in range(NCORES):
                    nc.sync.dma_start(out=qrawA[16 * s:16 * (s + 1), :],
                                      in_=h1view[i, s])
                ehqA = bpool.tile([DIM, 512], BF16, name="ehqA", tag="ehqA")
                nc.scalar.activation(ehqA[:], qrawA[:], EXP)
                pcs = psS.tile([DIM, 8], F32, name="pcs", tag="psb")
                for j in range(4):
                    nc.tensor.matmul(pcs[:, j:j + 1],
                                     ehqA[:, 128 * j:128 * (j + 1)],
                                     ones_col[:], start=True, stop=True)
                recipA = bpool.tile([DIM, 4], F32, name="recipA", tag="recipA")
                nc.vector.reciprocal(recipA[:], pcs[:, 0:4])
                nc.vector.tensor_scalar_mul(recipA[:, 0:2], recipA[:, 0:2],
                                            SCALE)
                recipQK = bpool.tile([64, 8], F32, name="recipQK",
                                     tag="recipQK")
                for blk in range(8):
                    nc.sync.dma_start(
                        out=recipQK[:, blk:blk + 1],
                        in_=recipA[64 * (blk % 2):64 * (blk % 2) + 64,
                                   blk // 2:blk // 2 + 1])
                qkT = []
                for blk in range(8):
                    s = bpool.tile([64, N], BF16, name=f"qkT{blk}",
                                   tag=f"qkT{blk}")
                    for n0, nsz in NH:
                        p = psD.tile([64, 512], F32, name="pproj", tag="pd")
                        nc.tensor.matmul(p[:, :nsz],
                                         ehqA[:, 64 * blk:64 * (blk + 1)],
                                         xT_sb[i][:, n0:n0 + nsz],
                                         start=True, stop=True)
                        nc.scalar.mul(s[:, n0:n0 + nsz], p[:, :nsz],
                                      recipQK[:, blk:blk + 1])
                    qkT.append(s)
                # dots / exp / mask / rowsum for every head (v not needed)
                em_all = {}
                rrs_all = {}
                for h in range(HEADS):
                    tq, rq = h // 2, 32 * (h % 2)
                    qt_t, kt_t = qkT[tq], qkT[4 + tq]
                    for hf, (n0, nsz) in enumerate(NH):
                        prs = psS.tile([1, 512], F32, name="prs", tag="prs")
                        e_ts, em_ts = [], []
                        for mt, (m0, msz) in enumerate(MT):
                            pd = psD.tile([128, 512], F32, name="pdots",
                                          tag="pd")
                            nc.tensor.matmul(pd[:msz, :nsz],
                                             kt_t[rq:rq + 32, m0:m0 + msz],
                                             qt_t[rq:rq + 32, n0:n0 + nsz],
                                             start=True, stop=True)
                            e_t = epool.tile([128, 300], BF16, name="e_t",
                                             tag="e")
                            nc.scalar.activation(e_t[:msz, :nsz],
                                                 pd[:msz, :nsz], EXP)
                            em_t = em_pool.tile([128, 300], BF16, name="em_t",
                                                tag="em")
                            nc.vector.tensor_mul(
                                em_t[:msz, :nsz], e_t[:msz, :nsz],
                                maskT_sb[i][mt][:msz, n0:n0 + nsz])
                            e_ts.append(e_t)
                            em_ts.append(em_t)
                        for mt, (m0, msz) in enumerate(MT):
                            nc.tensor.matmul(prs[:, :nsz], ones_col[:msz, :],
                                             e_ts[mt][:msz, :nsz],
                                             start=(mt == 0), stop=(mt == 4))
                        rrs = rrs_pool.tile([1, 300], BF16, name="rrs",
                                            tag="rrs")
                        with nc.allow_low_precision("attn normalizer bf16"):
                            nc.vector.reciprocal(rrs[:, :nsz], prs[:, :nsz])
                        em_all[(h, hf)] = em_ts
                        rrs_all[(h, hf)] = rrs
                p1_state[i] = (em_all, rrs_all)

            for i in range(BPC):
                em_all, rrs_all = p1_state[i]
                # ---------- part 2: needs a2a #2 (v / ow) ----------
                qrawB = bpool.tile([DIM, 256], BF16, name="qrawB", tag="qrawB")
                for s in range(NCORES):
                    nc.sync.dma_start(out=qrawB[16 * s:16 * (s + 1), :],
                                      in_=h2view[i, s])
                ehqB = bpool.tile([DIM, 256], BF16, name="ehqB", tag="ehqB")
                nc.scalar.activation(ehqB[:], qrawB[:], EXP)
                ehv = []
                for s in range(NCORES):
                    vr = bpool.tile([32, DIM], BF16, name=f"vraw{s}",
                                    tag=f"vraw{s}")
                    nc.sync.dma_start(out=vr[:], in_=hvview[i, s])
                    ev = bpool.tile([32, DIM], BF16, name=f"ehv{s}",
                                    tag=f"ehv{s}")
                    nc.scalar.activation(ev[:], vr[:], EXP)
                    ehv.append(ev)
                pcs2 = psS.tile([DIM, 8], F32, name="pcs2", tag="psb")
                for j in range(2):
                    nc.tensor.matmul(pcs2[:, j:j + 1],
                                     ehqB[:, 128 * j:128 * (j + 1)],
                                     ones_col[:], start=True, stop=True)
                for s in range(NCORES):
                    nc.tensor.matmul(pcs2[:, 2:3], ehv[s][:], ones_col[:32, :],
                                     start=(s == 0), stop=(s == NCORES - 1))
                recipB = bpool.tile([DIM, 4], F32, name="recipB", tag="recipB")
                nc.vector.reciprocal(recipB[:, 0:3], pcs2[:, 0:3])
                rv = bpool.tile([32, 8], F32, name="rv", tag="rv")
                for s in range(NCORES):
                    nc.sync.dma_start(
                        out=rv[:, s:s + 1],
                        in_=recipB[32 * (s % 4):32 * (s % 4) + 32,
                                   s // 4:s // 4 + 1])
                for s in range(NCORES):
                    nc.vector.tensor_scalar_mul(ehv[s][:], ehv[s][:],
                                                rv[:, s:s + 1])
                v_sb = []
                for mt, (m0, msz) in enumerate(MT):
                    p = psD.tile([DIM, 512], F32, name="pv", tag="pd")
                    nc.tensor.matmul(p[:msz, :INNER], xT_sb[i][:, m0:m0 + msz],
                                     ehqB[:], start=True, stop=True)
                    s = bpool.tile([128, INNER], BF16, name=f"v_sb{mt}",
                                   tag=f"v_sb{mt}")
                    nc.scalar.copy(s[:msz, :], p[:msz, :INNER])
                    v_sb.append(s)

                outT = [bpool.tile([32, N], BF16, name=f"outT{h}",
                                   tag=f"outT{h}") for h in range(HEADS)]
                for h in range(HEADS):
                    for hf, (n0, nsz) in enumerate(NH):
                        po = psO.tile([32, 512], F32, name="po", tag="po")
                        em_ts = em_all[(h, hf)]
                        for mt, (m0, msz) in enumerate(MT):
                            nc.tensor.matmul(po[:, :nsz],
                                             v_sb[mt][:msz, 32 * h:32 * h + 32],
                                             em_ts[mt][:msz, :nsz],
                                             start=(mt == 0), stop=(mt == 4))
                        pb = psS.tile([32, 512], F32, name="pb", tag="psb")
                        nc.tensor.matmul(pb[:, :nsz], ones_row32[:],
                                         rrs_all[(h, hf)][:, :nsz],
                                         start=True, stop=True)
                        ob = epool.tile([32, 300], BF16, name="ob", tag="ob")
                        nc.vector.tensor_copy(ob[:, :nsz], po[:, :nsz])
                        nc.vector.tensor_mul(outT[h][:, n0:n0 + nsz],
                                             ob[:, :nsz], pb[:, :nsz])

                # y: [dout=128, n=600] = sum_h ehv[h].T @ outT[h]
                ys = bpool.tile([DIM, N], F32, name="ys", tag="ys")
                for n0, nsz in NH:
                    py = psD.tile([DIM, 512], F32, name="py", tag="pd")
                    for h in range(HEADS):
                        nc.tensor.matmul(py[:, :nsz], ehv[h][:],
                                         outT[h][:, n0:n0 + nsz],
                                         start=(h == 0), stop=(h == HEADS - 1))
                    nc.scalar.mul(ys[:, n0:n0 + nsz], py[:, :nsz],
                                  recipB[:, 2:3])
                nc.sync.dma_start(out=yT[i], in_=ys[:])
            emctx.__exit__(None, None, None)

    return nc


_PROGRAM_CACHE = {}


def _get_program(with_bias):
    if with_bias not in _PROGRAM_CACHE:
        _PROGRAM_CACHE[with_bias] = _build_program(with_bias)
    return _PROGRAM_CACHE[with_bias]


def _shard_inputs(x, mask, resolution, framerate,
                  W1, b1, W2, b2, W3, b3, V1, c1, V2, c2, V3, c3, with_bias):
    b12, c12, b3f, c3f = with_bias
    x = np.asarray(x, np.float32)
    mask = np.asarray(mask, np.float32)
    xT = np.ascontiguousarray(x.transpose(0, 2, 1)).astype(BF16_NP)
    maskT = np.ascontiguousarray(
        mask[0, :, 0].transpose(0, 2, 1)).astype(BF16_NP)
    frateT = np.ascontiguousarray(
        np.stack([np.asarray(framerate, np.float32),
                  np.asarray(resolution, np.float32)], axis=0))
    W1 = np.ascontiguousarray(np.asarray(W1, np.float32))
    W2 = np.ascontiguousarray(np.asarray(W2, np.float32))
    V1 = np.ascontiguousarray(np.asarray(V1, np.float32))
    V2 = np.ascontiguousarray(np.asarray(V2, np.float32))
    W3v = np.asarray(W3, np.float32).reshape(D3, DIM, E3)
    V3v = np.asarray(V3, np.float32).reshape(INNER, INNER, DIM)
    in_maps = []
    for c in range(NCORES):
        m = {
            "xT": xT[BPC * c:BPC * (c + 1)],
            "maskT": maskT[BPC * c:BPC * (c + 1)],
            "frateT": frateT,
            "w1": W1, "w2": W2, "v1": V1, "v2": V2,
            # reordered: (d-major, e<512) then (d-major, e>=512) — matches
            # the split-a2a chunk layout in the device program
            "w3c": (np.concatenate([
                W3v[:, 16 * c:16 * (c + 1), :512].reshape(D3, 16 * 512),
                W3v[:, 16 * c:16 * (c + 1), 512:].reshape(D3, 16 * 256),
            ], axis=1) * W3_SCALE).astype(FP8_NP),
            "v3c": (np.ascontiguousarray(
                V3v[:, 32 * c:32 * (c + 1), :]).reshape(INNER, V3_SL)
                * W3_SCALE).astype(FP8_NP),
        }
        if b12:
            m["b1t"] = np.asarray(b1, np.float32).reshape(D3, 1)
            m["b2t"] = np.asarray(b2, np.float32).reshape(D3, 1)
        if c12:
            m["c1t"] = np.asarray(c1, np.float32).reshape(INNER, 1)
            m["c2t"] = np.asarray(c2, np.float32).reshape(INNER, 1)
        if b3f:
            b3v = np.asarray(b3, np.float32).reshape(DIM, E3)[16 * c:16 * (c + 1)]
            m["b3c"] = (np.concatenate(
                [b3v[:, :512].reshape(1, 16 * 512),
                 b3v[:, 512:].reshape(1, 16 * 256)], axis=1)
                * (W3_SCALE * A2_SCALE)).astype(BF16_NP)
        if c3f:
            m["c3c"] = (np.ascontiguousarray(
                np.asarray(c3, np.float32).reshape(INNER, DIM)
                [32 * c:32 * (c + 1)].reshape(1, V3_SL))
                * (W3_SCALE * A2_SCALE)).astype(BF16_NP)
        in_maps.append(m)
    return in_maps


def _run(inputs, trace=False, tmpdir=None):
    with_bias = (
        bool(np.any(inputs["b1"])) or bool(np.any(inputs["b2"])),
        bool(np.any(inputs["c1"])) or bool(np.any(inputs["c2"])),
        bool(np.any(inputs["b3"])),
        bool(np.any(inputs["c3"])),
    )
    nc = _get_program(with_bias)
    in_maps = _shard_inputs(with_bias=with_bias, **inputs)
    res = run_bass_kernel_spmd(nc, in_maps, core_ids=list(range(NCORES)),
                               trace=trace, tmpdir=tmpdir)
    outs = []
    for c in range(NCORES):
        yt = res.results[c]["yT"]  # [2, 128, 600]
        outs.append(yt.transpose(0, 2, 1))  # [2, 600, 128]
    full = np.ascontiguousarray(np.concatenate(outs, axis=0)).astype(np.float32)
    return full, res


def kernel(**inputs) -> np.ndarray:
    out, _ = _run(inputs, trace=False)
    return out
